# revision 7
# baseline (speedup 1.0000x reference)
"""Trainium2 Bass kernel for the MAB dense-transformer block.

Per-batch computation (B=8 batches -> 8 NeuronCores, pure data parallel):
  q = Q @ Wq.T + bq ; k = K @ Wk.T + bk ; v = K @ Wv.T + bv
  per 16 heads (dh=64): A = softmax(qh kh^T / 32) ; o = qh + A @ vh
  O = LN0(o) ; O = O + relu(O @ Wo.T + bo) ; O = LN1(O)

Device-side layout is fully transposed ([feature, seq]); the host wrapper
pre-transposes inputs and re-transposes the output (outside HW exec time).
All matmuls run in fp32r (TF32) at 1 cycle/row.  The attention residual,
fc residual, and both LNs are computed in place on the qT tiles, so the
whole block needs one persistent [D, S] activation array plus kT/vpack.
"""
import math
from contextlib import ExitStack

import numpy as np

import concourse.bass as bass
import concourse.tile as tile
from concourse import bacc, bass_isa, mybir
from concourse import bass_utils

F32 = mybir.dt.float32
F32R = mybir.dt.float32r
AF = mybir.ActivationFunctionType
OP = mybir.AluOpType

P = 128          # partitions / tile rows
D = 1024         # model dim (= DQ = DK = DV)
S = 1024         # sequence length (SQ = SK)
H = 16           # heads
DH = 64          # head dim
NT = D // P      # 8 feature tiles
CH = 512         # free-dim chunk (one psum bank of fp32)
NCH = S // CH    # 2 chunks
SCALE = 1.0 / math.sqrt(D)   # 1/32, folded into exp
EPS = 1e-5
NCORES = 8


def _f32(ap):
    return ap.bitcast(F32)


def _layernorm(nc, lnp, rowp, bcp, tiles, g_cols, b_cols):
    """In-place LN over the partition (feature) axis of 8 [P, S] tiles."""
    for c in range(NCH):
        cs = bass.ts(c, CH)
        acc = lnp.tile([P, CH], F32, name="lnacc", tag="lnacc")
        sq = lnp.tile([P, CH], F32, name="lnsq", tag="lnsq")
        sqt = lnp.tile([P, CH], F32, name="lnsqt", tag="lnsqt")
        # sums and sums-of-squares over the 8 tiles (gpsimd; DVE is busier)
        nc.gpsimd.tensor_add(acc[:], _f32(tiles[0][:, cs]), _f32(tiles[1][:, cs]))
        for t in range(2, NT):
            nc.gpsimd.tensor_add(acc[:], acc[:], _f32(tiles[t][:, cs]))
        nc.gpsimd.tensor_mul(sq[:], _f32(tiles[0][:, cs]), _f32(tiles[0][:, cs]))
        for t in range(1, NT):
            nc.gpsimd.tensor_mul(sqt[:], _f32(tiles[t][:, cs]), _f32(tiles[t][:, cs]))
            nc.gpsimd.tensor_add(sq[:], sq[:], sqt[:])
        nc.gpsimd.partition_all_reduce(acc[:], acc[:], channels=P,
                                       reduce_op=bass_isa.ReduceOp.add)
        nc.gpsimd.partition_all_reduce(sq[:], sq[:], channels=P,
                                       reduce_op=bass_isa.ReduceOp.add)
        # row stats: mean, msq in place on partition 0 of acc/sq
        nc.vector.tensor_scalar_mul(acc[0:1, :], acc[0:1, :], 1.0 / D)
        nc.vector.tensor_scalar_mul(sq[0:1, :], sq[0:1, :], 1.0 / D)
        r1 = rowp.tile([1, CH], F32, name="lnr1", tag="lnr1")
        r2 = rowp.tile([1, CH], F32, name="lnr2", tag="lnr2")
        nc.vector.tensor_mul(r1[:], acc[0:1, :], acc[0:1, :])
        nc.vector.tensor_sub(r1[:], sq[0:1, :], r1[:])
        nc.vector.tensor_scalar_add(r1[:], r1[:], EPS)
        nc.scalar.activation(r1[:], r1[:], AF.Sqrt)
        nc.vector.reciprocal(out=r1[:], in_=r1[:])          # rstd
        nc.vector.tensor_mul(r2[:], acc[0:1, :], r1[:])
        nc.vector.tensor_scalar_mul(r2[:], r2[:], -1.0)     # -mean*rstd
        ab = bcp.tile([P, CH], F32, name="lnab", tag="lnab")
        bb = bcp.tile([P, CH], F32, name="lnbb", tag="lnbb")
        nc.gpsimd.partition_broadcast(ab[:], r1[:])
        nc.gpsimd.partition_broadcast(bb[:], r2[:])
        # apply: x = (x*rstd - mean*rstd) * g + b
        for t in range(NT):
            nc.vector.tensor_mul(tiles[t][:, cs], _f32(tiles[t][:, cs]), ab[:])
            nc.vector.tensor_add(tiles[t][:, cs], _f32(tiles[t][:, cs]), bb[:])
            nc.vector.tensor_scalar(tiles[t][:, cs], _f32(tiles[t][:, cs]),
                                    g_cols[:, t:t + 1], b_cols[:, t:t + 1],
                                    OP.mult, OP.add)


def build():
    nc = bacc.Bacc("TRN2", target_bir_lowering=False, debug=False,
                   enable_asserts=False, num_devices=NCORES)
    QT = nc.dram_tensor("QT", [D, S], F32R, kind="ExternalInput").ap()
    KT = nc.dram_tensor("KT", [D, S], F32R, kind="ExternalInput").ap()
    WqT = nc.dram_tensor("WqT", [D, D], F32R, kind="ExternalInput").ap()
    WkT = nc.dram_tensor("WkT", [D, D], F32R, kind="ExternalInput").ap()
    WvT = nc.dram_tensor("WvT", [D, D], F32R, kind="ExternalInput").ap()
    WoT = nc.dram_tensor("WoT", [D, D], F32R, kind="ExternalInput").ap()
    bq = nc.dram_tensor("bq", [D], F32, kind="ExternalInput").ap()
    bk = nc.dram_tensor("bk", [D], F32, kind="ExternalInput").ap()
    bv = nc.dram_tensor("bv", [D], F32, kind="ExternalInput").ap()
    bo = nc.dram_tensor("bo", [D], F32, kind="ExternalInput").ap()
    g0 = nc.dram_tensor("g0", [D], F32, kind="ExternalInput").ap()
    b0 = nc.dram_tensor("b0", [D], F32, kind="ExternalInput").ap()
    g1 = nc.dram_tensor("g1", [D], F32, kind="ExternalInput").ap()
    b1 = nc.dram_tensor("b1", [D], F32, kind="ExternalInput").ap()
    ones = nc.dram_tensor("ones", [P, H], F32R, kind="ExternalInput").ap()
    OUT = nc.dram_tensor("OUT", [D, S], F32, kind="ExternalOutput").ap()

    with tile.TileContext(nc) as tc, ExitStack() as ctx:
        consts = ctx.enter_context(tc.tile_pool(name="consts", bufs=1))

        def col_load(name, src):
            t = consts.tile([P, NT], F32, name=name, tag=name)
            nc.sync.dma_start(out=t[:], in_=src.rearrange("(t p) -> p t", p=P))
            return t
        bq_c = col_load("bq_c", bq)
        bk_c = col_load("bk_c", bk)
        bo_c = col_load("bo_c", bo)
        g0_c = col_load("g0_c", g0)
        b0_c = col_load("b0_c", b0)
        g1_c = col_load("g1_c", g1)
        b1_c = col_load("b1_c", b1)
        bv_row = consts.tile([1, D], F32, name="bv_row", tag="bv_row")
        nc.sync.dma_start(out=bv_row[:], in_=bv[None, :])
        bvb = consts.tile([P, D], F32, name="bvb", tag="bvb")
        nc.gpsimd.partition_broadcast(bvb[:], bv_row[:])

        # persistent activations: qT doubles as attention output, LN0 input,
        # fc input, fc-residual output, LN1 input and the final result.
        pp = ctx.enter_context(tc.tile_pool(name="pp", bufs=1))
        qTt = [pp.tile([P, S], F32R, name=f"qT{t}", tag=f"qT{t}") for t in range(NT)]
        kTt = [pp.tile([P, S], F32R, name=f"kT{t}", tag=f"kT{t}") for t in range(NT)]
        vp = [pp.tile([P, H * 65], F32R, name=f"vp{t}", tag=f"vp{t}") for t in range(NT)]
        for t in range(NT):
            ov = vp[t][:].rearrange("p (h c) -> p h c", h=H)[:, :, 64:65]
            nc.sync.dma_start(out=ov, in_=ones[:, :, None])

        # ---- Phase A: projections ----
        with ExitStack() as pab:
            psA = pab.enter_context(tc.tile_pool(name="psA", bufs=4, space="PSUM"))
            with tc.tile_pool(name="kts", bufs=1) as ktsp:
                kts = [ktsp.tile([P, S], F32R, name=f"kts{k}", tag=f"kts{k}")
                       for k in range(NT)]
                for k in range(NT):
                    nc.sync.dma_start(out=kts[k][:], in_=KT[bass.ts(k, P), :])
                with tc.tile_pool(name="wk", bufs=1) as wkp:
                    wk = [wkp.tile([P, D], F32R, name=f"wk{k}", tag=f"wk{k}")
                          for k in range(NT)]
                    for k in range(NT):
                        nc.sync.dma_start(out=wk[k][:], in_=WkT[bass.ts(k, P), :])
                    for m in range(NT):
                        for c in range(NCH):
                            ps = psA.tile([P, CH], F32, name="psa", tag="psa")
                            for k in range(NT):
                                nc.tensor.matmul(ps[:], wk[k][:, bass.ts(m, P)],
                                                 kts[k][:, bass.ts(c, CH)],
                                                 start=(k == 0), stop=(k == NT - 1))
                            nc.scalar.add(kTt[m][:, bass.ts(c, CH)], ps[:],
                                          bk_c[:, m:m + 1])
                with tc.tile_pool(name="wv", bufs=1) as wvp:
                    wv = [wvp.tile([P, D], F32R, name=f"wv{k}", tag=f"wv{k}")
                          for k in range(NT)]
                    for k in range(NT):
                        nc.sync.dma_start(out=wv[k][:], in_=WvT[bass.ts(k, P), :])
                    for m in range(NT):
                        for c in range(NCH):
                            ps = psA.tile([P, CH], F32, name="psa", tag="psa")
                            for k in range(NT):
                                nc.tensor.matmul(ps[:], kts[k][:, bass.ts(m, P)],
                                                 wv[k][:, bass.ts(c, CH)],
                                                 start=(k == 0), stop=(k == NT - 1))
                            # strided drain into vpack (+bv), heads 8c..8c+7
                            hview = vp[m][:].rearrange("p (h c) -> p h c", h=H)
                            outv = hview[:, 8 * c:8 * c + 8, 0:64]
                            psv = ps[:].rearrange("p (h c) -> p h c", h=8)
                            bvv = bvb[:, bass.ts(c, CH)].rearrange(
                                "p (h c) -> p h c", h=8)
                            nc.vector.tensor_add(outv, psv, bvv)
            with tc.tile_pool(name="qts", bufs=1) as qtsp:
                qts = [qtsp.tile([P, S], F32R, name=f"qts{k}", tag=f"qts{k}")
                       for k in range(NT)]
                for k in range(NT):
                    nc.sync.dma_start(out=qts[k][:], in_=QT[bass.ts(k, P), :])
                with tc.tile_pool(name="wq", bufs=1) as wqp:
                    wq = [wqp.tile([P, D], F32R, name=f"wq{k}", tag=f"wq{k}")
                          for k in range(NT)]
                    for k in range(NT):
                        nc.sync.dma_start(out=wq[k][:], in_=WqT[bass.ts(k, P), :])
                    for m in range(NT):
                        for c in range(NCH):
                            ps = psA.tile([P, CH], F32, name="psa", tag="psa")
                            for k in range(NT):
                                nc.tensor.matmul(ps[:], wq[k][:, bass.ts(m, P)],
                                                 qts[k][:, bass.ts(c, CH)],
                                                 start=(k == 0), stop=(k == NT - 1))
                            nc.scalar.add(qTt[m][:, bass.ts(c, CH)], ps[:],
                                          bq_c[:, m:m + 1])

        # ---- Phase B: attention, head pairs row-packed; residual in place ----
        with tc.tile_pool(name="scps", bufs=4, space="PSUM") as scps, \
             tc.tile_pool(name="avps", bufs=4, space="PSUM") as avps, \
             tc.tile_pool(name="expp", bufs=12) as expp, \
             tc.tile_pool(name="smp", bufs=2) as smp:
            for c in range(NCH):
                cs = bass.ts(c, CH)
                for r in range(NT):
                    ha, hb = 2 * r, 2 * r + 1
                    ea, eb = [], []
                    for t in range(NT):
                        sca = scps.tile([P, CH], F32, name="sc", tag="sc")
                        nc.tensor.matmul(sca[:], kTt[r][0:64, bass.ts(t, P)],
                                         qTt[r][0:64, cs], start=True, stop=True)
                        scb = scps.tile([P, CH], F32, name="sc", tag="sc")
                        nc.tensor.matmul(scb[:], kTt[r][64:128, bass.ts(t, P)],
                                         qTt[r][64:128, cs], start=True, stop=True)
                        et = expp.tile([P, CH], F32R, name="exp", tag="exp")
                        nc.scalar.activation(et[:], sca[:], AF.Exp, scale=SCALE)
                        ea.append(et)
                        et = expp.tile([P, CH], F32R, name="exp", tag="exp")
                        nc.scalar.activation(et[:], scb[:], AF.Exp, scale=SCALE)
                        eb.append(et)
                    ava = avps.tile([65, CH], F32, name="av", tag="av")
                    avb = avps.tile([65, CH], F32, name="av", tag="av")
                    for t in range(NT):
                        nc.tensor.matmul(ava[:], vp[t][:, ha * 65:ha * 65 + 65],
                                         ea[t][:], start=(t == 0), stop=(t == NT - 1))
                        nc.tensor.matmul(avb[:], vp[t][:, hb * 65:hb * 65 + 65],
                                         eb[t][:], start=(t == 0), stop=(t == NT - 1))
                    for hp, av in ((0, ava), (1, avb)):
                        hs = slice(hp * 64, hp * 64 + 64)
                        rd = smp.tile([1, CH], F32, name="rd", tag="rd")
                        nc.vector.reciprocal(out=rd[:], in_=av[64:65, :])
                        rdb = smp.tile([64, CH], F32, name="rdb", tag="rdb")
                        nc.gpsimd.partition_broadcast(rdb[:], rd[:])
                        tmp = smp.tile([P, CH], F32, name="tmp", tag="tmp")
                        nc.vector.tensor_mul(tmp[0:64, :], av[0:64, :], rdb[:])
                        if hp == 1:
                            # engines are partition-lane-tied: hop the
                            # normalized rows to base 64 via SBUF->SBUF DMA
                            nc.sync.dma_start(out=tmp[64:128, :], in_=tmp[0:64, :])
                        nc.vector.tensor_add(qTt[r][hs, cs], tmp[hs, :],
                                             _f32(qTt[r][hs, cs]))

        # ---- Phases C/D: LN0, fc_o + relu + residual (in place), LN1 ----
        with tc.tile_pool(name="lnp", bufs=2) as lnp, \
             tc.tile_pool(name="rowp", bufs=2) as rowp, \
             tc.tile_pool(name="bcp", bufs=2) as bcp:
            _layernorm(nc, lnp, rowp, bcp, qTt, g0_c, b0_c)
            with tc.tile_pool(name="wo", bufs=1) as wop, \
                 tc.tile_pool(name="rp", bufs=4) as rp, \
                 tc.tile_pool(name="psD", bufs=4, space="PSUM") as psD:
                wo = [wop.tile([P, D], F32R, name=f"wo{k}", tag=f"wo{k}")
                      for k in range(NT)]
                for k in range(NT):
                    nc.sync.dma_start(out=wo[k][:], in_=WoT[bass.ts(k, P), :])
                for c in range(NCH):
                    cs = bass.ts(c, CH)
                    for m in range(NT):
                        ps = psD.tile([P, CH], F32, name="psd", tag="psd")
                        for k in range(NT):
                            nc.tensor.matmul(ps[:], wo[k][:, bass.ts(m, P)],
                                             qTt[k][:, cs],
                                             start=(k == 0), stop=(k == NT - 1))
                        rt = rp.tile([P, CH], F32, name="rt", tag="rt")
                        nc.scalar.activation(rt[:], ps[:], AF.Relu,
                                             bias=bo_c[:, m:m + 1])
                        # kTt is dead after attention; reuse it as the fc
                        # output so qTt (the fc rhs) is never overwritten
                        nc.vector.tensor_add(kTt[m][:, cs], rt[:],
                                             _f32(qTt[m][:, cs]))
            _layernorm(nc, lnp, rowp, bcp, kTt, g1_c, b1_c)
            for t in range(NT):
                nc.sync.dma_start(out=OUT[bass.ts(t, P), :], in_=_f32(kTt[t][:]))
    nc.compile()
    return nc


_NC_CACHE = None


def _get_nc():
    global _NC_CACHE
    if _NC_CACHE is None:
        _NC_CACHE = build()
    return _NC_CACHE


def make_in_maps(Q, K, Wq, bq, Wk, bk, Wv, bv, Wo, bo, g0, b0, g1, b1):
    WqT = np.ascontiguousarray(np.asarray(Wq, np.float32).T)
    WkT = np.ascontiguousarray(np.asarray(Wk, np.float32).T)
    WvT = np.ascontiguousarray(np.asarray(Wv, np.float32).T)
    WoT = np.ascontiguousarray(np.asarray(Wo, np.float32).T)
    ones = np.ones((P, H), np.float32)
    common = dict(WqT=WqT, WkT=WkT, WvT=WvT, WoT=WoT,
                  bq=np.asarray(bq, np.float32), bk=np.asarray(bk, np.float32),
                  bv=np.asarray(bv, np.float32), bo=np.asarray(bo, np.float32),
                  g0=np.asarray(g0, np.float32), b0=np.asarray(b0, np.float32),
                  g1=np.asarray(g1, np.float32), b1=np.asarray(b1, np.float32),
                  ones=ones)
    in_maps = []
    for b in range(NCORES):
        in_maps.append(dict(common,
                            QT=np.ascontiguousarray(np.asarray(Q[b], np.float32).T),
                            KT=np.ascontiguousarray(np.asarray(K[b], np.float32).T)))
    return in_maps


def kernel(Q, K, Wq, bq, Wk, bk, Wv, bv, Wo, bo, g0, b0, g1, b1):
    nc = _get_nc()
    in_maps = make_in_maps(Q, K, Wq, bq, Wk, bk, Wv, bv, Wo, bo, g0, b0, g1, b1)
    res = bass_utils.run_bass_kernel_spmd(nc, in_maps, core_ids=list(range(NCORES)))
    out = np.empty((NCORES, S, D), np.float32)
    for b in range(NCORES):
        out[b] = res.results[b]["OUT"].T
    return out
